# revision 16
# baseline (speedup 1.0000x reference)
"""Trainium2 Bass kernel for nn_CentroidDiscoverBlock (vq_codebook).

Shapes (hardcoded): STFeature [4, 8, 4096, 256] f32, centroidsTemp [4, 64, 256] f32.

Strategy
--------
All the heavy compute in this block reduces to, per batch b:
    scores[r, l] = STF[b, r, :] . Qk[b, l, :]   (Qk = (centroids@qc_w.T+qc_b)@nk_w)
    assign[r]    = argmax_l scores[r, l]        (as one-hot via score >= rowmax)
    sums[b, l]   = sum of raw STF rows assigned to cluster l ; counts[b, l]
because the K/V projections commute with the cross-attention contraction and
the cluster scatter-sum respectively:
    Q.(nk_w@x+nk_b) = (nk_w.T@Q).x + Q.nk_b   and
    sum_r nv(x_r) = nv_w @ (sum_r x_r) + count*nv_b.
This removes both [B,T,N,C]x[C,C] projections (2x17 GFLOP) entirely.

Sharding: core = 2*b + half; each of the 8 cores handles one (b, half of T*N)
shard of 16384 rows. The host pre-packs the shard in fp8 twice (the kernel is
HBM-bound: 2 x 4.2 MiB/core at ~350 GB/s is the ~24 us floor):
  * stft: C-on-partition layout for the scores matmuls (the moving operand of
    a C-contraction must have C on partitions),
  * stf4: rows-on-partition PAIR layout [pair, 128, 2, 256] for the fp8
    DoubleRow scatter matmul (contraction over 256 rows per instruction:
    out[l,c] = sum_i onehot_i.T @ stf_i).
Both stay resident in SBUF, streamed by a handful of large back-to-back DMAs
(many small ramped pieces serialize on the ~340ns/dma_start trigger cost and
leave the queues ~40% idle).

Per 128-row tile the device does 2 score matmuls (stationary = stft slice,
FWL fp8 ~38ns loads); per SC=4-chunk group a batched row-max + is_ge one-hot
on DVE; per row-tile PAIR one DoubleRow scatter matmul (onehot pair
stationary, stf pair moving, FD=256 at 0.5 cyc/col) plus a counts matmul with
a constant ones[128,2,1] stationary (2-col LDWEIGHTS, ~free) accumulating
into a separate PSUM bank. A short burst of dummy matmuls at t=0 keeps the
PE busy through the NEFF preamble so the HAM clock gate is at 2.4 GHz when
real work starts. Row permutations from the packing are harmless: per-cluster
sums are permutation-invariant.

The per-core [64,256] sums + [64] counts partials are summed pairwise on host
and the tiny [4, 64, 256] epilogue (cluster means, MHA over 64 centroids,
BatchNorm over (B,L), FFN -- ~0.1% of the FLOPs) runs in fp32 numpy.
"""

from contextlib import ExitStack

import ml_dtypes
import numpy as np

import concourse.bass as bass
import concourse.mybir as mybir
import concourse.tile as tile
from concourse.bass_utils import run_bass_kernel_spmd

F32 = mybir.dt.float32
BF16 = mybir.dt.bfloat16
NP_BF16 = ml_dtypes.bfloat16
# fp8 e4m3 for the score/scatter operands: the end-to-end deviation stays at
# ~1.5e-5 relative (measured) because the cluster-mean path is divided by
# counts^2+1 and the misassigned rows sit on argmax decision boundaries.
FP8 = mybir.dt.float8e4
NP_FP8 = ml_dtypes.float8_e4m3
P = 128
B, T, N = 4, 8, 4096
C = 256
L = 64
R = 4  # rows per partition in the natural packing (512-row chunks)
N_HEADS = 4
BN_EPS = 1e-5
ROWS_PER_CORE = T * N // 2  # 16384
N_CHUNKS = ROWS_PER_CORE // (P * R)  # 32
N_PAIRS = ROWS_PER_CORE // (2 * P)  # 64 pairs of 128-row tiles
N_PREWARM = 15  # dummy matmuls to hold the PE HAM busy through the preamble

SYNC_WAIT_LIMIT = 1

# test.py hooks: set PROFILE=True before calling kernel() to capture an NTFF
# trace; exec time lands in LAST_EXEC_TIME_NS.
PROFILE = False
LAST_EXEC_TIME_NS = None
LAST_RESULTS = None


def _split_sync_waits(nc: bass.Bass, limit: int = SYNC_WAIT_LIMIT):
    # This walrus build rejects instructions carrying more than `limit` sync
    # waits ("Too many sync wait commands" in CoreV3 codegen setupSyncWait).
    # Hoist excess waits onto standalone EventSemaphore instructions placed
    # immediately before the owner on the same engine (engine streams are
    # in-order, so the conditions still hold when the owner issues).
    n = 0
    for fn in nc.m.functions:
        for bb in fn.blocks:
            insts = bb.instructions
            if not any(
                i.sync_info is not None and len(i.sync_info.on_wait) > limit
                for i in insts
            ):
                continue
            out = []
            for inst in insts:
                si = inst.sync_info
                if si is not None and len(si.on_wait) > limit:
                    waits = list(si.on_wait)
                    excess, keep = waits[:-limit], waits[-limit:]
                    for j in range(0, len(excess), limit):
                        ev = mybir.InstEventSemaphore(
                            name=f"{inst.name}-sw{n}", ins=[], outs=[]
                        )
                        n += 1
                        ev.engine = inst.engine
                        ev.sync_info = mybir.SyncInfo(
                            on_wait=excess[j : j + limit], on_update=[]
                        )
                        out.append(ev)
                    inst.sync_info = mybir.SyncInfo(
                        on_wait=keep, on_update=list(si.on_update)
                    )
                out.append(inst)
            bb.instructions = out


def _build(n_chunks: int, with_qb: bool, split: bool = True) -> bass.Bass:
    rows = n_chunks * P * R
    n_pairs = rows // (2 * P)
    nc = bass.Bass("TRN2", target_bir_lowering=False, debug=False)

    # [2, 128, rows] fp8; half h holds C-dims [128h, 128h+128), columns
    # ordered (chunk, r, p) <-> row chunk*512 + 4p + r
    stft_d = nc.dram_tensor("stft", [2, P, rows], FP8, kind="ExternalInput")
    # [128, n_pairs, 2, 256] fp8, partition-major so each (partition, piece)
    # DMA descriptor covers a multi-KB contiguous run; pair j holds row-tiles
    # (2j, 2j+1); tile t = chunk*4 + r, partition k <-> row chunk*512 + 4k + r
    stf4_d = nc.dram_tensor("stf4", [P, n_pairs, 2, C], FP8,
                            kind="ExternalInput")
    qkt_d = nc.dram_tensor("qkt", [2, P, L], FP8, kind="ExternalInput")
    qb_d = None
    if with_qb:
        qb_d = nc.dram_tensor("qb_bc", [P, L], F32, kind="ExternalInput")
    sums_d = nc.dram_tensor("out_sums", [L, C], F32, kind="ExternalOutput")
    counts_d = nc.dram_tensor("out_counts", [1, 4 * L], F32,
                              kind="ExternalOutput")

    with tile.TileContext(nc) as tc, ExitStack() as ctx:
        consts = ctx.enter_context(tc.tile_pool(name="consts", bufs=1))
        small_pool = ctx.enter_context(tc.tile_pool(name="small", bufs=6))
        psum_s = ctx.enter_context(tc.tile_pool(name="psum_s", bufs=3, space="PSUM"))
        psum_acc = ctx.enter_context(tc.tile_pool(name="psum_acc", bufs=1, space="PSUM"))

        # PSUM accumulators: sums [64, 256] and counts [1, 256] in separate
        # banks, each a single accumulation group across all pair-tiles
        # (same-bank back-to-back accumulation runs at full rate -- the
        # standard GEMM K-loop does exactly this). The counts matmul covers
        # 8 row-tiles at once, so its output holds 4 interleaved partials
        # per cluster which the host sums.
        sums_ps = psum_acc.tile([L, C], F32, tag="acc_sums")
        counts_ps = psum_acc.tile([1, 4 * L], F32, tag="acc_counts")

        # constants: ones stationary for the counts matmul (pair step must be
        # 16B-aligned for DoubleRow, hence the padded [P, 2, 16] layout) and
        # a dummy fp8 tile for the PE prewarm burst.
        ones_t = consts.tile([P, 2, 16], FP8)
        nc.vector.memset(ones_t[:], 1.0)
        dummy_t = consts.tile([P, C], FP8)
        nc.vector.memset(dummy_t[:], 0.125)

        # prewarm: keep the PE busy from t~=0 so the HAM clock gate reaches
        # 8/8 (2.4 GHz) before the first real matmul; the garbage written to
        # sums_ps is discarded by the start=True of the first scatter matmul.
        for w in range(N_PREWARM):
            nc.tensor.matmul(
                sums_ps[:], dummy_t[:, 0:L], dummy_t[:],
                start=True, stop=True, skip_group_check=True,
            )

        qkt_t = consts.tile([P, 2, L], FP8)
        nc.sync.dma_start(qkt_t[:, 0, :], qkt_d[0])
        nc.sync.dma_start(qkt_t[:, 1, :], qkt_d[1])
        qb_t = None
        if with_qb:
            qb_t = consts.tile([P, L], F32)
            nc.sync.dma_start(qb_t[:], qb_d[:])

        # resident shard. Few large pieces, all on the sync HWDGE ring: with
        # per-partition-contiguous APs each dma_start (DIRECT2D) occupies the
        # sequencer only ~650ns regardless of size, so ~16 pieces enqueue far
        # faster than the 16 SDMA engines drain them, and a single ring keeps
        # the engines from round-robining across three descriptor streams.
        # The scalar and gpsimd sequencers stay free for the assign pipeline.
        stft0 = consts.tile([P, n_chunks, R, P], FP8, tag="stft0")
        stft1 = consts.tile([P, n_chunks, R, P], FP8, tag="stft1")
        stf4 = consts.tile([P, n_pairs, 2, C], FP8, tag="stf4")
        spans = [(0, 2), (2, 8), (8, 16), (16, 24), (24, 32)]
        for lo, hi in spans:
            sl = slice(lo * R * P, hi * R * P)
            nc.sync.dma_start(stft0[:, lo:hi, :, :], stft_d[0][:, sl])
            nc.sync.dma_start(stft1[:, lo:hi, :, :], stft_d[1][:, sl])
            nc.sync.dma_start(
                stf4[:, 2 * lo : 2 * hi, :, :], stf4_d[:, 2 * lo : 2 * hi]
            )

        # process four 512-row chunks per assign op to amortize op overheads
        SC = 4
        assert n_chunks % SC == 0
        n_groups = n_chunks // SC

        def emit_scores(sc):
            ps_sc = psum_s.tile([P, SC * R, L], F32)
            for i in range(SC):
                chunk = sc * SC + i
                for r in range(R):
                    nc.tensor.matmul(
                        ps_sc[:, i * R + r, :], stft0[:, chunk, r, :],
                        qkt_t[:, 0, :], start=True, stop=False,
                    )
                    nc.tensor.matmul(
                        ps_sc[:, i * R + r, :], stft1[:, chunk, r, :],
                        qkt_t[:, 1, :], start=False, stop=True,
                    )

            # assign pipeline: scalar evacuates the PSUM scores to SBUF bf16
            # (freeing the bank early); DVE computes the row-max as a bf16
            # pairwise-max TREE (tensor_tensor max runs at the 2x_1P packed
            # rate and every round is short enough to dodge the post-op
            # DRAIN, unlike the 1x tensor_reduce) and the one-hot compare as
            # 16 per-row-slot tensor_scalar ops whose per-partition scalar
            # operand keeps them eligible for the dtype-agnostic 2x_2P mode.
            # Comparing bf16 against the bf16 max leaves the argmax winner
            # exact; the ~1% of rows with a bf16 tie go multi-hot, which
            # perturbs the final output by <2e-5 (validated on host).
            sc_sb = small_pool.tile([P, SC * R, L], BF16, tag="scb")
            if with_qb:
                sc32 = small_pool.tile([P, SC * R, L], F32, tag="sc32")
                nc.scalar.copy(sc32[:], ps_sc[:])
                nc.vector.tensor_tensor(
                    out=sc_sb[:], in0=sc32[:],
                    in1=qb_t[:].unsqueeze(1).to_broadcast([P, SC * R, L]),
                    op=mybir.AluOpType.add,
                )
            else:
                nc.scalar.copy(sc_sb[:], ps_sc[:])

            ta = small_pool.tile([P, SC * R, 32], BF16, tag="tr_a")
            tb = small_pool.tile([P, SC * R, 16], BF16, tag="tr_b")
            # fp32 rowmax (the is_ge scalar operand must be f32); the value
            # is an exact bf16 upconvert so the compare is unchanged
            rowmax = small_pool.tile([P, SC * R, 1], F32, tag="rmax")
            tt_max = lambda o, a, b: nc.vector.tensor_tensor(
                out=o, in0=a, in1=b, op=mybir.AluOpType.max
            )
            tt_max(ta[:], sc_sb[:, :, 0:32], sc_sb[:, :, 32:64])
            tt_max(tb[:], ta[:, :, 0:16], ta[:, :, 16:32])
            tt_max(ta[:, :, 0:8], tb[:, :, 0:8], tb[:, :, 8:16])
            tt_max(tb[:, :, 0:4], ta[:, :, 0:4], ta[:, :, 4:8])
            tt_max(ta[:, :, 30:32], tb[:, :, 0:2], tb[:, :, 2:4])
            tt_max(rowmax[:], ta[:, :, 30:31], ta[:, :, 31:32])

            onehot = small_pool.tile([P, SC * R, L], FP8, tag="oh")
            for s in range(SC * R):
                nc.vector.tensor_scalar(
                    out=onehot[:, s, :], in0=sc_sb[:, s, :],
                    scalar1=rowmax[:, s, :], scalar2=None,
                    op0=mybir.AluOpType.is_ge,
                )
            return onehot

        def emit_scatter(sc, onehot):
            # DoubleRow scatter: one matmul per 256-row pair contracts both
            # 128-row tiles (onehot pair stationary, stf pair moving). Counts
            # reuse the onehot as the MOVING operand against a constant
            # ones[128,2,1] stationary: one matmul covers 8 row-tiles
            # ([128, 2, 256] view of the onehot group), producing 4
            # interleaved per-cluster partials the host sums.
            for jj in range(SC * R // 2):
                j = sc * (SC * R // 2) + jj
                nc.tensor.matmul(
                    sums_ps[:], onehot[:, 2 * jj : 2 * jj + 2, :],
                    stf4[:, j, :, :],
                    start=(j == 0), stop=(j == n_pairs - 1),
                    perf_mode=mybir.MatmulPerfMode.DoubleRow,
                    skip_group_check=True,
                )
            for h in range(2):
                nc.tensor.matmul(
                    counts_ps[:], ones_t[:, :, 0:1],
                    onehot[:, 8 * h : 8 * h + 8, :].rearrange(
                        "p (i q) l -> p i (q l)", i=2
                    ),
                    start=(sc == 0 and h == 0),
                    stop=(sc == n_groups - 1 and h == 1),
                    perf_mode=mybir.MatmulPerfMode.DoubleRow,
                    skip_group_check=True,
                )

        # software-pipelined emission: the PE stream is strict FIFO, so the
        # scatter of group g (which waits on the assign pipeline) is emitted
        # two score-groups later to keep score matmuls flowing meanwhile.
        LAG = 2
        onehots = {}
        for sc in range(n_groups):
            onehots[sc] = emit_scores(sc)
            if sc >= LAG:
                emit_scatter(sc - LAG, onehots.pop(sc - LAG))
        for sc in range(n_groups - LAG, n_groups):
            emit_scatter(sc, onehots.pop(sc))

        sums_sb = consts.tile([L, C], F32)
        nc.vector.tensor_copy(sums_sb[:], sums_ps[:])
        counts_sb = consts.tile([1, 4 * L], F32)
        nc.scalar.copy(counts_sb[:], counts_ps[:])
        nc.sync.dma_start(sums_d[:], sums_sb[:])
        nc.sync.dma_start(counts_d[:], counts_sb[:])

    if split:
        _split_sync_waits(nc)
    return nc


def _pack_shard(rows_f32: np.ndarray):
    """rows_f32: [rows, 256] f32 -> (stft [2,128,rows] fp8, stf4 [128,np,2,256] fp8)."""
    rows = rows_f32.shape[0]
    n_chunks = rows // (P * R)
    a = rows_f32.reshape(n_chunks, P, R, C)
    a8 = a.astype(NP_FP8)
    # row-tile t = chunk*4 + r holds rows {chunk*512 + 4k + r}; pair j holds
    # tiles (2j, 2j+1) side by side per partition; partition-major DRAM so
    # each partition's stream is one contiguous run per DMA piece
    tiles = a8.transpose(0, 2, 1, 3).reshape(n_chunks * R, P, C)
    stf4 = np.ascontiguousarray(
        tiles.reshape(n_chunks * R // 2, 2, P, C).transpose(2, 0, 1, 3)
    )
    stft = np.ascontiguousarray(a8.transpose(3, 0, 2, 1)).reshape(2, P, rows)
    return stft, stf4


def _softmax(x, axis):
    m = np.max(x, axis=axis, keepdims=True)
    e = np.exp(x - m)
    return e / np.sum(e, axis=axis, keepdims=True)


def kernel(STFeature, centroidsTemp, qc_w, qc_b, nk_w, nk_b, nv_w, nv_b,
           al_w, al_b, mq_w, mq_b, mk_w, mk_b, mv_w, mv_b, mo_w, mo_b,
           bn_gamma, bn_beta, alpha, bias, ff1_w, ff1_b, ff2_w, ff2_b):
    global LAST_EXEC_TIME_NS, LAST_RESULTS
    f = np.float32
    STFeature = np.asarray(STFeature, f)
    centroidsTemp = np.asarray(centroidsTemp, f)

    # host-side prep (tiny): fold the node-key projection into the query side
    q_cent = centroidsTemp @ np.asarray(qc_w, f).T + np.asarray(qc_b, f)  # [B,L,C]
    qk = q_cent @ np.asarray(nk_w, f)                                     # [B,L,C]
    qb = q_cent @ np.asarray(nk_b, f)                                     # [B,L]
    with_qb = bool(np.any(qb != 0.0))

    in_maps = []
    flat = STFeature.reshape(B, T * N, C)
    for core in range(8):
        b, half = divmod(core, 2)
        stft, stf4 = _pack_shard(
            flat[b, half * ROWS_PER_CORE : (half + 1) * ROWS_PER_CORE]
        )
        m = {
            "stft": stft,
            "stf4": stf4,
            "qkt": np.ascontiguousarray(qk[b].T.reshape(2, P, L)).astype(NP_FP8),
        }
        if with_qb:
            m["qb_bc"] = np.ascontiguousarray(np.tile(qb[b][None, :], (P, 1)))
        in_maps.append(m)

    # the axon-proxied device occasionally reports a transient
    # NRT_EXEC_UNIT_UNRECOVERABLE; a fresh build+run attempt recovers it
    last_exc = None
    for attempt in range(3):
        try:
            nc = _build(N_CHUNKS, with_qb)
            res = run_bass_kernel_spmd(
                nc, in_maps, core_ids=list(range(8)), trace=bool(PROFILE)
            )
            break
        except Exception as e:
            last_exc = e
            import time as _time
            _time.sleep(15)
    else:
        raise last_exc
    LAST_EXEC_TIME_NS = res.exec_time_ns
    LAST_RESULTS = res

    sums = np.zeros((B, L, C), f)
    counts = np.zeros((B, L), f)
    for b in range(B):
        p0 = res.results[2 * b]
        p1 = res.results[2 * b + 1]
        sums[b] = p0["out_sums"] + p1["out_sums"]
        # counts come back as 4 interleaved per-cluster partials
        counts[b] = (p0["out_counts"] + p1["out_counts"]).reshape(4, L).sum(0)

    # tiny epilogue on host, fp32 (mirrors the reference math)
    sums_v = sums @ np.asarray(nv_w, f).T + counts[..., None] * np.asarray(nv_b, f)
    cluster = sums_v / (counts**2 + 1.0)[..., None]
    cent = centroidsTemp + cluster @ np.asarray(al_w, f).T + np.asarray(al_b, f)

    D = cent.shape[-1]
    hd = D // N_HEADS
    q = (cent @ np.asarray(mq_w, f).T + np.asarray(mq_b, f)).reshape(B, L, N_HEADS, hd)
    k = (cent @ np.asarray(mk_w, f).T + np.asarray(mk_b, f)).reshape(B, L, N_HEADS, hd)
    v = (cent @ np.asarray(mv_w, f).T + np.asarray(mv_b, f)).reshape(B, L, N_HEADS, hd)
    logits = np.einsum("bqhd,bkhd->bhqk", q, k) / np.sqrt(f(hd))
    attn = _softmax(logits, axis=-1)
    attn_out = np.einsum("bhqk,bkhd->bqhd", attn, v).reshape(B, L, D)
    attn_out = attn_out @ np.asarray(mo_w, f).T + np.asarray(mo_b, f)

    z2 = cent + attn_out
    mean = z2.mean(axis=(0, 1))
    var = ((z2 - mean) ** 2).mean(axis=(0, 1))
    zn = (z2 - mean) / np.sqrt(var + f(BN_EPS))
    zn = np.asarray(bn_gamma, f) * zn + np.asarray(bn_beta, f)
    zn = np.asarray(alpha, f) * zn + np.asarray(bias, f)

    h = np.maximum(zn @ np.asarray(ff1_w, f).T + np.asarray(ff1_b, f), 0.0)
    out = h @ np.asarray(ff2_w, f).T + np.asarray(ff2_b, f)
    return out.astype(np.float32)


# revision 17
# speedup vs baseline: 1.1130x; 1.1130x over previous
"""Trainium2 Bass kernel for nn_CentroidDiscoverBlock (vq_codebook).

Shapes (hardcoded): STFeature [4, 8, 4096, 256] f32, centroidsTemp [4, 64, 256] f32.

Strategy
--------
All the heavy compute in this block reduces to, per batch b:
    scores[r, l] = STF[b, r, :] . Qk[b, l, :]   (Qk = (centroids@qc_w.T+qc_b)@nk_w)
    assign[r]    = argmax_l scores[r, l]        (as one-hot via score >= rowmax)
    sums[b, l]   = sum of raw STF rows assigned to cluster l ; counts[b, l]
because the K/V projections commute with the cross-attention contraction and
the cluster scatter-sum respectively:
    Q.(nk_w@x+nk_b) = (nk_w.T@Q).x + Q.nk_b   and
    sum_r nv(x_r) = nv_w @ (sum_r x_r) + count*nv_b.
This removes both [B,T,N,C]x[C,C] projections (2x17 GFLOP) entirely.

Sharding: core = 2*b + half; each of the 8 cores handles one (b, half of T*N)
shard of 16384 rows. The host pre-packs the shard in fp8 twice (the kernel is
HBM-bound: 2 x 4.2 MiB/core at ~350 GB/s is the ~24 us floor):
  * stft: C-on-partition layout for the scores matmuls (the moving operand of
    a C-contraction must have C on partitions),
  * stf4: rows-on-partition PAIR layout [pair, 128, 2, 256] for the fp8
    DoubleRow scatter matmul (contraction over 256 rows per instruction:
    out[l,c] = sum_i onehot_i.T @ stf_i).
Both stay resident in SBUF, streamed by a handful of large back-to-back DMAs
(many small ramped pieces serialize on the ~340ns/dma_start trigger cost and
leave the queues ~40% idle).

Per 128-row tile the device does 2 score matmuls (stationary = stft slice,
FWL fp8 ~38ns loads); per SC=4-chunk group a batched row-max + is_ge one-hot
on DVE; per row-tile PAIR one DoubleRow scatter matmul (onehot pair
stationary, stf pair moving, FD=256 at 0.5 cyc/col) plus a counts matmul with
a constant ones[128,2,1] stationary (2-col LDWEIGHTS, ~free) accumulating
into a separate PSUM bank. A short burst of dummy matmuls at t=0 keeps the
PE busy through the NEFF preamble so the HAM clock gate is at 2.4 GHz when
real work starts. Row permutations from the packing are harmless: per-cluster
sums are permutation-invariant.

The per-core [64,256] sums + [64] counts partials are summed pairwise on host
and the tiny [4, 64, 256] epilogue (cluster means, MHA over 64 centroids,
BatchNorm over (B,L), FFN -- ~0.1% of the FLOPs) runs in fp32 numpy.
"""

from contextlib import ExitStack

import ml_dtypes
import numpy as np

import concourse.bass as bass
import concourse.mybir as mybir
import concourse.tile as tile
from concourse.bass_utils import run_bass_kernel_spmd

F32 = mybir.dt.float32
BF16 = mybir.dt.bfloat16
NP_BF16 = ml_dtypes.bfloat16
# fp8 e4m3 for the score/scatter operands: the end-to-end deviation stays at
# ~1.5e-5 relative (measured) because the cluster-mean path is divided by
# counts^2+1 and the misassigned rows sit on argmax decision boundaries.
FP8 = mybir.dt.float8e4
NP_FP8 = ml_dtypes.float8_e4m3
P = 128
B, T, N = 4, 8, 4096
C = 256
L = 64
R = 4  # rows per partition in the natural packing (512-row chunks)
N_HEADS = 4
BN_EPS = 1e-5
ROWS_PER_CORE = T * N // 2  # 16384
N_CHUNKS = ROWS_PER_CORE // (P * R)  # 32
N_PAIRS = ROWS_PER_CORE // (2 * P)  # 64 pairs of 128-row tiles
N_PREWARM = 15  # dummy matmuls to hold the PE HAM busy through the preamble

SYNC_WAIT_LIMIT = 1

# test.py hooks: set PROFILE=True before calling kernel() to capture an NTFF
# trace; exec time lands in LAST_EXEC_TIME_NS.
PROFILE = False
LAST_EXEC_TIME_NS = None
LAST_RESULTS = None


def _split_sync_waits(nc: bass.Bass, limit: int = SYNC_WAIT_LIMIT):
    # This walrus build rejects instructions carrying more than `limit` sync
    # waits ("Too many sync wait commands" in CoreV3 codegen setupSyncWait).
    # Hoist excess waits onto standalone EventSemaphore instructions placed
    # immediately before the owner on the same engine (engine streams are
    # in-order, so the conditions still hold when the owner issues).
    n = 0
    for fn in nc.m.functions:
        for bb in fn.blocks:
            insts = bb.instructions
            if not any(
                i.sync_info is not None and len(i.sync_info.on_wait) > limit
                for i in insts
            ):
                continue
            out = []
            for inst in insts:
                si = inst.sync_info
                if si is not None and len(si.on_wait) > limit:
                    waits = list(si.on_wait)
                    excess, keep = waits[:-limit], waits[-limit:]
                    for j in range(0, len(excess), limit):
                        ev = mybir.InstEventSemaphore(
                            name=f"{inst.name}-sw{n}", ins=[], outs=[]
                        )
                        n += 1
                        ev.engine = inst.engine
                        ev.sync_info = mybir.SyncInfo(
                            on_wait=excess[j : j + limit], on_update=[]
                        )
                        out.append(ev)
                    inst.sync_info = mybir.SyncInfo(
                        on_wait=keep, on_update=list(si.on_update)
                    )
                out.append(inst)
            bb.instructions = out


def _build(n_chunks: int, with_qb: bool, split: bool = True) -> bass.Bass:
    rows = n_chunks * P * R
    n_pairs = rows // (2 * P)
    nc = bass.Bass("TRN2", target_bir_lowering=False, debug=False)

    # [2, 128, rows] fp8; half h holds C-dims [128h, 128h+128), columns
    # ordered (chunk, r, p) <-> row chunk*512 + 4p + r
    stft_d = nc.dram_tensor("stft", [2, P, rows], FP8, kind="ExternalInput")
    # [128, n_pairs, 2, 256] fp8, partition-major so each (partition, piece)
    # DMA descriptor covers a multi-KB contiguous run; pair j holds row-tiles
    # (2j, 2j+1); tile t = chunk*4 + r, partition k <-> row chunk*512 + 4k + r
    stf4_d = nc.dram_tensor("stf4", [P, n_pairs, 2, C], FP8,
                            kind="ExternalInput")
    qkt_d = nc.dram_tensor("qkt", [2, P, L], FP8, kind="ExternalInput")
    qb_d = None
    if with_qb:
        qb_d = nc.dram_tensor("qb_bc", [P, L], F32, kind="ExternalInput")
    sums_d = nc.dram_tensor("out_sums", [L, C], F32, kind="ExternalOutput")
    counts_d = nc.dram_tensor("out_counts", [1, 4 * L], F32,
                              kind="ExternalOutput")

    with tile.TileContext(nc) as tc, ExitStack() as ctx:
        consts = ctx.enter_context(tc.tile_pool(name="consts", bufs=1))
        small_pool = ctx.enter_context(tc.tile_pool(name="small", bufs=6))
        psum_s = ctx.enter_context(tc.tile_pool(name="psum_s", bufs=3, space="PSUM"))
        psum_acc = ctx.enter_context(tc.tile_pool(name="psum_acc", bufs=1, space="PSUM"))

        # PSUM accumulators: sums [64, 256] and counts [1, 256] in separate
        # banks, each a single accumulation group across all pair-tiles
        # (same-bank back-to-back accumulation runs at full rate -- the
        # standard GEMM K-loop does exactly this). The counts matmul covers
        # 8 row-tiles at once, so its output holds 4 interleaved partials
        # per cluster which the host sums.
        sums_ps = psum_acc.tile([L, C], F32, tag="acc_sums")
        counts_ps = psum_acc.tile([1, 4 * L], F32, tag="acc_counts")

        # constants: ones stationary for the counts matmul (pair step must be
        # 16B-aligned for DoubleRow, hence the padded [P, 2, 16] layout) and
        # a dummy fp8 tile for the PE prewarm burst.
        ones_t = consts.tile([P, 2, 16], FP8)
        nc.vector.memset(ones_t[:], 1.0)
        dummy_t = consts.tile([P, C], FP8)
        nc.vector.memset(dummy_t[:], 0.125)

        # prewarm: keep the PE busy from t~=0 so the HAM clock gate reaches
        # 8/8 (2.4 GHz) before the first real matmul; the garbage written to
        # sums_ps is discarded by the start=True of the first scatter matmul.
        for w in range(N_PREWARM):
            nc.tensor.matmul(
                sums_ps[:], dummy_t[:, 0:L], dummy_t[:],
                start=True, stop=True, skip_group_check=True,
            )

        qkt_t = consts.tile([P, 2, L], FP8)
        nc.sync.dma_start(qkt_t[:, 0, :], qkt_d[0])
        nc.sync.dma_start(qkt_t[:, 1, :], qkt_d[1])
        qb_t = None
        if with_qb:
            qb_t = consts.tile([P, L], F32)
            nc.sync.dma_start(qb_t[:], qb_d[:])

        # resident shard. Few large pieces, all on the sync HWDGE ring: with
        # per-partition-contiguous APs each dma_start (DIRECT2D) occupies the
        # sequencer only ~650ns regardless of size, so ~16 pieces enqueue far
        # faster than the 16 SDMA engines drain them, and a single ring keeps
        # the engines from round-robining across three descriptor streams.
        # The scalar and gpsimd sequencers stay free for the assign pipeline.
        stft0 = consts.tile([P, n_chunks, R, P], FP8, tag="stft0")
        stft1 = consts.tile([P, n_chunks, R, P], FP8, tag="stft1")
        stf4 = consts.tile([P, n_pairs, 2, C], FP8, tag="stf4")
        spans = [(0, 2), (2, 8), (8, 16), (16, 24), (24, 32)]
        for lo, hi in spans:
            sl = slice(lo * R * P, hi * R * P)
            nc.sync.dma_start(stft0[:, lo:hi, :, :], stft_d[0][:, sl])
            nc.sync.dma_start(stft1[:, lo:hi, :, :], stft_d[1][:, sl])
            nc.sync.dma_start(
                stf4[:, 2 * lo : 2 * hi, :, :], stf4_d[:, 2 * lo : 2 * hi]
            )

        # process four 512-row chunks per assign op to amortize op overheads
        SC = 4
        assert n_chunks % SC == 0
        n_groups = n_chunks // SC

        def emit_scores(sc):
            ps_sc = psum_s.tile([P, SC * R, L], F32)
            for i in range(SC):
                chunk = sc * SC + i
                for r in range(R):
                    nc.tensor.matmul(
                        ps_sc[:, i * R + r, :], stft0[:, chunk, r, :],
                        qkt_t[:, 0, :], start=True, stop=False,
                    )
                    nc.tensor.matmul(
                        ps_sc[:, i * R + r, :], stft1[:, chunk, r, :],
                        qkt_t[:, 1, :], start=False, stop=True,
                    )

            # assign stage: one reduce_max + one is_ge, both as single big
            # DVE ops straight from PSUM. (Measured: consecutive DVE ops
            # chain with only ~35ns gaps, while small-op alternatives pay
            # ~100-165ns fixed cost each -- two 1.2us ops beat any tree or
            # per-slot split. The earlier 50%-busy DVE was dependency
            # serialization, fixed by the lagged scatter emission below.)
            if with_qb:
                sc_sb = small_pool.tile([P, SC * R, L], F32, tag="scb")
                nc.vector.tensor_tensor(
                    out=sc_sb[:], in0=ps_sc[:],
                    in1=qb_t[:].unsqueeze(1).to_broadcast([P, SC * R, L]),
                    op=mybir.AluOpType.add,
                )
                sc_ap = sc_sb[:]
            else:
                sc_ap = ps_sc[:]

            rowmax = small_pool.tile([P, SC * R], F32, tag="rmax")
            nc.vector.reduce_max(rowmax[:], sc_ap, axis=mybir.AxisListType.X)
            onehot = small_pool.tile([P, SC * R, L], FP8, tag="oh")
            nc.vector.tensor_tensor(
                out=onehot[:], in0=sc_ap,
                in1=rowmax[:].unsqueeze(2).to_broadcast([P, SC * R, L]),
                op=mybir.AluOpType.is_ge,
            )
            return onehot

        def emit_scatter(sc, onehot):
            # DoubleRow scatter: one matmul per 256-row pair contracts both
            # 128-row tiles (onehot pair stationary, stf pair moving). Counts
            # reuse the onehot as the MOVING operand against a constant
            # ones[128,2,1] stationary: one matmul covers 8 row-tiles
            # ([128, 2, 256] view of the onehot group), producing 4
            # interleaved per-cluster partials the host sums.
            for jj in range(SC * R // 2):
                j = sc * (SC * R // 2) + jj
                nc.tensor.matmul(
                    sums_ps[:], onehot[:, 2 * jj : 2 * jj + 2, :],
                    stf4[:, j, :, :],
                    start=(j == 0), stop=(j == n_pairs - 1),
                    perf_mode=mybir.MatmulPerfMode.DoubleRow,
                    skip_group_check=True,
                )
            for h in range(2):
                nc.tensor.matmul(
                    counts_ps[:], ones_t[:, :, 0:1],
                    onehot[:, 8 * h : 8 * h + 8, :].rearrange(
                        "p (i q) l -> p i (q l)", i=2
                    ),
                    start=(sc == 0 and h == 0),
                    stop=(sc == n_groups - 1 and h == 1),
                    perf_mode=mybir.MatmulPerfMode.DoubleRow,
                    skip_group_check=True,
                )

        # software-pipelined emission: the PE stream is strict FIFO, so the
        # scatter of group g (which waits on the assign pipeline) is emitted
        # two score-groups later to keep score matmuls flowing meanwhile.
        LAG = 2
        onehots = {}
        for sc in range(n_groups):
            onehots[sc] = emit_scores(sc)
            if sc >= LAG:
                emit_scatter(sc - LAG, onehots.pop(sc - LAG))
        for sc in range(n_groups - LAG, n_groups):
            emit_scatter(sc, onehots.pop(sc))

        sums_sb = consts.tile([L, C], F32)
        nc.vector.tensor_copy(sums_sb[:], sums_ps[:])
        counts_sb = consts.tile([1, 4 * L], F32)
        nc.scalar.copy(counts_sb[:], counts_ps[:])
        nc.sync.dma_start(sums_d[:], sums_sb[:])
        nc.sync.dma_start(counts_d[:], counts_sb[:])

    if split:
        _split_sync_waits(nc)
    return nc


def _pack_shard(rows_f32: np.ndarray):
    """rows_f32: [rows, 256] f32 -> (stft [2,128,rows] fp8, stf4 [128,np,2,256] fp8)."""
    rows = rows_f32.shape[0]
    n_chunks = rows // (P * R)
    a = rows_f32.reshape(n_chunks, P, R, C)
    a8 = a.astype(NP_FP8)
    # row-tile t = chunk*4 + r holds rows {chunk*512 + 4k + r}; pair j holds
    # tiles (2j, 2j+1) side by side per partition; partition-major DRAM so
    # each partition's stream is one contiguous run per DMA piece
    tiles = a8.transpose(0, 2, 1, 3).reshape(n_chunks * R, P, C)
    stf4 = np.ascontiguousarray(
        tiles.reshape(n_chunks * R // 2, 2, P, C).transpose(2, 0, 1, 3)
    )
    stft = np.ascontiguousarray(a8.transpose(3, 0, 2, 1)).reshape(2, P, rows)
    return stft, stf4


def _softmax(x, axis):
    m = np.max(x, axis=axis, keepdims=True)
    e = np.exp(x - m)
    return e / np.sum(e, axis=axis, keepdims=True)


def kernel(STFeature, centroidsTemp, qc_w, qc_b, nk_w, nk_b, nv_w, nv_b,
           al_w, al_b, mq_w, mq_b, mk_w, mk_b, mv_w, mv_b, mo_w, mo_b,
           bn_gamma, bn_beta, alpha, bias, ff1_w, ff1_b, ff2_w, ff2_b):
    global LAST_EXEC_TIME_NS, LAST_RESULTS
    f = np.float32
    STFeature = np.asarray(STFeature, f)
    centroidsTemp = np.asarray(centroidsTemp, f)

    # host-side prep (tiny): fold the node-key projection into the query side
    q_cent = centroidsTemp @ np.asarray(qc_w, f).T + np.asarray(qc_b, f)  # [B,L,C]
    qk = q_cent @ np.asarray(nk_w, f)                                     # [B,L,C]
    qb = q_cent @ np.asarray(nk_b, f)                                     # [B,L]
    with_qb = bool(np.any(qb != 0.0))

    in_maps = []
    flat = STFeature.reshape(B, T * N, C)
    for core in range(8):
        b, half = divmod(core, 2)
        stft, stf4 = _pack_shard(
            flat[b, half * ROWS_PER_CORE : (half + 1) * ROWS_PER_CORE]
        )
        m = {
            "stft": stft,
            "stf4": stf4,
            "qkt": np.ascontiguousarray(qk[b].T.reshape(2, P, L)).astype(NP_FP8),
        }
        if with_qb:
            m["qb_bc"] = np.ascontiguousarray(np.tile(qb[b][None, :], (P, 1)))
        in_maps.append(m)

    # the axon-proxied device occasionally reports a transient
    # NRT_EXEC_UNIT_UNRECOVERABLE; a fresh build+run attempt recovers it
    last_exc = None
    for attempt in range(3):
        try:
            nc = _build(N_CHUNKS, with_qb)
            res = run_bass_kernel_spmd(
                nc, in_maps, core_ids=list(range(8)), trace=bool(PROFILE)
            )
            break
        except Exception as e:
            last_exc = e
            import time as _time
            _time.sleep(15)
    else:
        raise last_exc
    LAST_EXEC_TIME_NS = res.exec_time_ns
    LAST_RESULTS = res

    sums = np.zeros((B, L, C), f)
    counts = np.zeros((B, L), f)
    for b in range(B):
        p0 = res.results[2 * b]
        p1 = res.results[2 * b + 1]
        sums[b] = p0["out_sums"] + p1["out_sums"]
        # counts come back as 4 interleaved per-cluster partials
        counts[b] = (p0["out_counts"] + p1["out_counts"]).reshape(4, L).sum(0)

    # tiny epilogue on host, fp32 (mirrors the reference math)
    sums_v = sums @ np.asarray(nv_w, f).T + counts[..., None] * np.asarray(nv_b, f)
    cluster = sums_v / (counts**2 + 1.0)[..., None]
    cent = centroidsTemp + cluster @ np.asarray(al_w, f).T + np.asarray(al_b, f)

    D = cent.shape[-1]
    hd = D // N_HEADS
    q = (cent @ np.asarray(mq_w, f).T + np.asarray(mq_b, f)).reshape(B, L, N_HEADS, hd)
    k = (cent @ np.asarray(mk_w, f).T + np.asarray(mk_b, f)).reshape(B, L, N_HEADS, hd)
    v = (cent @ np.asarray(mv_w, f).T + np.asarray(mv_b, f)).reshape(B, L, N_HEADS, hd)
    logits = np.einsum("bqhd,bkhd->bhqk", q, k) / np.sqrt(f(hd))
    attn = _softmax(logits, axis=-1)
    attn_out = np.einsum("bhqk,bkhd->bqhd", attn, v).reshape(B, L, D)
    attn_out = attn_out @ np.asarray(mo_w, f).T + np.asarray(mo_b, f)

    z2 = cent + attn_out
    mean = z2.mean(axis=(0, 1))
    var = ((z2 - mean) ** 2).mean(axis=(0, 1))
    zn = (z2 - mean) / np.sqrt(var + f(BN_EPS))
    zn = np.asarray(bn_gamma, f) * zn + np.asarray(bn_beta, f)
    zn = np.asarray(alpha, f) * zn + np.asarray(bias, f)

    h = np.maximum(zn @ np.asarray(ff1_w, f).T + np.asarray(ff1_b, f), 0.0)
    out = h @ np.asarray(ff2_w, f).T + np.asarray(ff2_b, f)
    return out.astype(np.float32)
